# revision 25
# baseline (speedup 1.0000x reference)
"""CrossSessionCenterAlignMarginLoss — Trainium2 Bass kernel (8 NeuronCores).

Math notes
----------
reference computes, with g_i = 2*label_i + session_i (4 groups):
    counts_j, sums_j = segment_sum over features           -> centers_j = sums_j/counts_j
    center = mean_i (1 - cos(f_i, c_{g_i}))
    align  = ((1-cos(c0,c1)) + (1-cos(c2,c3))) / 2
    margin = mean_{a in {0,1}, b in {2,3}} cos(c_a, c_b)
    total  = center + 0.1*align + 0.05*margin

Per-sample cosines collapse: cos(f_i, c_j) = dot(f_i/|f_i|, c_j)/|c_j|, so
    sum_{i in group j} cos(f_i, c_j) = dot(t_j, c_j) / |c_j|
where t_j = segment_sum of row-normalized features.  The device kernel only
needs ONE pass over features, producing (4,D) `sums` and (4,D) `t` per core:

  per 128-row tile:  rownorm r_i = 1/sqrt(sum_d f_id^2)   (ACT square+accum)
                     lhsT = [onehot | onehot * r]  (128 x 8)
                     psum += lhsT.T @ f_tile        (PE, fp32)

Data-parallel over B across the 8 cores; host reduces the 8 tiny (8,D)
partials and evaluates the scalar loss terms in float64.
"""

import numpy as np

import concourse.bacc as bacc
import concourse.bass as bass
import concourse.tile as tile
from concourse import mybir
from concourse.bass_utils import run_bass_kernel_spmd

B, D = 16384, 2048
NCORES = 8
BL = B // NCORES          # rows per core: 2048
P = 128                   # partitions
KT = BL // P              # K-tiles per core: 16
NCHUNK = 512              # matmul moving free dim (one PSUM bank, fp32)
NCH = D // NCHUNK         # 4
EPS = 1e-8

# set by test harness to capture a profile
TRACE = False
LAST_EXEC_NS = None
LAST_TRACE_PATH = None

_NC_CACHE = {}


def _build_nc():
    nc = bacc.Bacc("TRN2", target_bir_lowering=False)
    f_in = nc.dram_tensor("f", [BL, D], mybir.dt.float32r, kind="ExternalInput")
    g_in = nc.dram_tensor("g", [P, KT * 4], mybir.dt.float32, kind="ExternalInput")
    out = nc.dram_tensor("out", [8, D], mybir.dt.float32, kind="ExternalOutput")

    f_t = f_in[:].rearrange("(t p) d -> t p d", p=P)

    with tile.TileContext(nc) as tc:
        with (
            tc.tile_pool(name="ftiles", bufs=8) as fpool,
            tc.tile_pool(name="scratch", bufs=1) as scratch,
            tc.tile_pool(name="small", bufs=3) as small,
            tc.tile_pool(name="singles", bufs=1) as singles,
            tc.tile_pool(name="psum", bufs=1, space="PSUM") as psum,
        ):
            # start streaming the first feature tiles before anything else
            f_tiles = {}
            for t in range(2):
                f_tiles[t] = fpool.tile([P, D], mybir.dt.float32r, name="f_tile", tag="f_tile")
                nc.sync.dma_start(out=f_tiles[t][:], in_=f_t[t])

            # one-hot groups, host-packed as [P, KT, 4]
            g_sb = singles.tile([P, KT, 4], mybir.dt.float32)
            nc.sync.dma_start(out=g_sb[:], in_=g_in[:].rearrange("p (t c) -> p t c", c=4))

            psum_acc = [
                psum.tile([8, NCHUNK], mybir.dt.float32, name=f"acc{n}")
                for n in range(NCH)
            ]

            for t in range(KT):
                if t in f_tiles:
                    f_tile = f_tiles[t]
                else:
                    # f is pre-rounded to fp32r on the host: a pure move
                    f_tile = fpool.tile([P, D], mybir.dt.float32r, name="f_tile", tag="f_tile")
                    nc.sync.dma_start(out=f_tile[:], in_=f_t[t])

                # row sum-of-squares in one ACT pass (sq is scratch)
                sq = scratch.tile([P, D], mybir.dt.float32, tag="sq")
                ssq = small.tile([P, 1], mybir.dt.float32, tag="ssq")
                nc.scalar.activation(
                    out=sq[:], in_=f_tile[:].bitcast(mybir.dt.float32),
                    func=mybir.ActivationFunctionType.Square,
                    accum_out=ssq[:],
                )
                nrm = small.tile([P, 1], mybir.dt.float32, tag="nrm")
                nc.scalar.sqrt(nrm[:], ssq[:])
                r = small.tile([P, 1], mybir.dt.float32, tag="r")
                nc.vector.reciprocal(r[:], nrm[:])

                # lhsT = [onehot | onehot * (1/|f|)]  (tiny DVE ops, fp32r out)
                lhsT = small.tile([P, 8], mybir.dt.float32r, tag="lhsT")
                nc.vector.tensor_copy(lhsT[:, 0:4], g_sb[:, t, :])
                nc.vector.tensor_scalar_mul(lhsT[:, 4:8], g_sb[:, t, :], r[:])

                # fp32r streams 1 row/cycle on PE (plain fp32 takes 4)
                for n in range(NCH):
                    nc.tensor.matmul(
                        psum_acc[n][:],
                        lhsT[:],
                        f_tile[:, n * NCHUNK:(n + 1) * NCHUNK],
                        start=(t == 0),
                        stop=(t == KT - 1),
                    )

            out_sb = singles.tile([8, D], mybir.dt.float32)
            for n in range(NCH):
                # alternate engines so the four drain copies run in parallel
                eng = nc.vector if n % 2 == 0 else nc.scalar
                if eng is nc.vector:
                    nc.vector.tensor_copy(out_sb[:, n * NCHUNK:(n + 1) * NCHUNK], psum_acc[n][:])
                else:
                    nc.scalar.copy(out_sb[:, n * NCHUNK:(n + 1) * NCHUNK], psum_acc[n][:])
            nc.sync.dma_start(out=out[:], in_=out_sb[:])

    nc.compile()
    return nc


def _get_nc():
    if "nc" not in _NC_CACHE:
        _NC_CACHE["nc"] = _build_nc()
    return _NC_CACHE["nc"]


def _round_fp32r(x):
    """Round fp32 to the PE's fp32r format: 11 mantissa bits kept, round
    half to even on the 12 dropped bits (matches walrus fp32_to_fp32r)."""
    b = x.view(np.uint32)
    low = b & np.uint32(0xFFF)
    keep = (b & np.uint32(0xFFFFF000)).astype(np.uint64)
    lsb = (b >> np.uint32(12)) & np.uint32(1)
    up = (low > 0x800) | ((low == 0x800) & (lsb == 1))
    keep += up.astype(np.uint64) << np.uint64(12)
    return (keep & np.uint64(0xFFFFFFFF)).astype(np.uint32).view(np.float32)


def _cos(a, b):
    num = float(np.dot(a, b))
    den = max(float(np.linalg.norm(a) * np.linalg.norm(b)), EPS)
    return num / den


def kernel(features, labels, sessions):
    global LAST_EXEC_NS, LAST_TRACE_PATH
    feats = np.ascontiguousarray(np.asarray(features), dtype=np.float32)
    feats = _round_fp32r(feats)
    labels = np.asarray(labels).astype(np.int64)
    sessions = np.asarray(sessions).astype(np.int64)
    g = labels * 2 + sessions                      # (B,) in 0..3

    onehot = np.zeros((B, 4), np.float32)
    onehot[np.arange(B), g] = 1.0
    counts = np.bincount(g, minlength=4).astype(np.float64)

    in_maps = []
    for c in range(NCORES):
        fl = feats[c * BL:(c + 1) * BL]
        ol = onehot[c * BL:(c + 1) * BL]
        # pack [BL,4] -> [P, KT*4]: partition p, tile t -> row t*P+p
        ol = np.ascontiguousarray(
            ol.reshape(KT, P, 4).transpose(1, 0, 2).reshape(P, KT * 4)
        )
        in_maps.append({"f": np.ascontiguousarray(fl), "g": ol})

    nc = _get_nc()
    res = run_bass_kernel_spmd(nc, in_maps, core_ids=list(range(NCORES)), trace=TRACE)
    if TRACE:
        LAST_EXEC_NS = res.exec_time_ns
        LAST_TRACE_PATH = (res.instructions_and_trace or (None, None))[1]

    acc = np.zeros((8, D), np.float64)
    for rmap in res.results:
        acc += rmap["out"].astype(np.float64)
    S = acc[0:4]         # segment sums of raw features
    T = acc[4:8]         # segment sums of normalized features

    centers = S / counts[:, None]
    cn = np.linalg.norm(centers, axis=1)

    sum_cos = sum(
        float(np.dot(T[j], centers[j])) / max(cn[j], EPS) for j in range(4)
    )
    center_loss = 1.0 - sum_cos / B

    align_loss = ((1.0 - _cos(centers[0], centers[1]))
                  + (1.0 - _cos(centers[2], centers[3]))) / 2.0
    margin_loss = np.mean([
        _cos(centers[a], centers[b]) for a in (0, 1) for b in (2, 3)
    ])
    total = 1.0 * center_loss + 0.1 * align_loss + 0.05 * margin_loss

    return np.array([total, center_loss, align_loss, margin_loss], dtype=np.float32)
